# revision 25
# baseline (speedup 1.0000x reference)
"""Multi-head causal self-attention on 8 Trainium2 NeuronCores.

Problem: B=4, S=2048, D=1024, H=16 heads (Dh=64), fp32, causal + key-padding
mask, out = softmax(mask(QK^T/sqrt(Dh))) V Wo^T with Q/K/V = x @ W*^T.

Sharding (data-parallel over batch x tensor-parallel over heads):
  core = 2*b + g  (b in 0..3, g in 0..1): batch b, head group g (8 heads).
  Each core computes its 8 heads' attention and a partial output projection
  through its row-slice of Wo; the host sums the two partials per batch
  (the "all-reduce" of the hint, done on host since outputs are gathered
  anyway).

Per-core kernel, one software-pipelined loop over 512-column s-chunks:
  - All matmul operands are bf16 (psum accumulation f32): same PE rate as
    fp32r at >=256-wide output, but no 4x penalty on the narrow diagonal
    matmuls, half the input DMA, and the 16-bit DVE fast path for the
    triangular mask multiply.
  - Causality makes attention q-chunk c depend only on projection chunks
    0..c, so the schedule is: proj(0); attn(0)+proj(1); attn(1)+proj(2)+
    Wo(0); attn(2)+proj(3)+Wo(1); attn(3)+Wo(2); Wo(3).
  - The attention inner loop is Activation-(exp-)bound: per 128x512 score
    tile the PE owes 426ns (QK^T + AV) while exp costs ~550ns. Projection /
    output-projection matmuls are therefore injected one instruction at a
    time (generator fillers) after each AV, and scores are emitted
    LOOKAHEAD tiles ahead of their AVs so the in-order PE queue never
    blocks on exp latency.
  - scores computed TRANSPOSED per head: s^T[k, q] = k^T_tile.T @ q^T so the
    softmaxed tile feeds the AV matmul directly as the moving operand.
  - V carries an appended ones-column per head so the AV matmul also yields
    the softmax denominators (row 64 of the [65, q] psum tile).
  - normalize: reciprocal on DVE, partition-broadcast on GpSimd, multiply on
    DVE straight into ctx^T tiles, the stationary operand of the output
    projection out[s, d] = ctx^T.T @ Wo_slice^T.
"""

import numpy as np

import concourse.bass as bass
import concourse.mybir as mybir
import concourse.tile as tile
from concourse import bacc
from concourse.bass_utils import run_bass_kernel_spmd

P = 128
NEG = -1.0e30
LOOKAHEAD = 2


def _round_f32r(a: np.ndarray) -> np.ndarray:
    """Round fp32 values to the PE's fp32r grid (11-bit mantissa,
    round-half-to-even at bit 12)."""
    bits = np.ascontiguousarray(a, dtype=np.float32).view(np.uint32)
    low = bits & np.uint32(0xFFF)
    hi = bits & np.uint32(0xFFFFF000)
    add = (low > 0x800) | ((low == 0x800) & (((bits >> 12) & 1) == 1))
    return (hi + (add.astype(np.uint32) << 12)).view(np.float32)


class Cfg:
    def __init__(self, B=4, S=2048, D=1024, H=16, Dh=64, n_cores=8, qch=512,
                 mm_dtype="bf16", reps=1):
        self.reps = reps
        self.B, self.S, self.D, self.H, self.Dh = B, S, D, H, Dh
        self.n_cores = n_cores
        self.groups = n_cores // B              # head groups (tensor-parallel)
        self.Hc = H // self.groups              # heads per core
        self.F = self.Hc * Dh                   # per-core q/k/v feature width
        self.qch = qch                          # q columns per score matmul
        self.nqc = S // qch                     # q chunks
        self.qt_per_ch = qch // P               # 128-row q tiles per chunk
        self.nt_s = S // P                      # key/seq tiles
        self.nt_d = D // P                      # contraction tiles (D)
        self.nt_f = self.F // P                 # feature tiles
        self.heads_per_ft = P // Dh             # heads packed per feature tile
        self.mm_dtype = mm_dtype

    @property
    def mdt(self):
        return {"fp32r": mybir.dt.float32r,
                "fp32": mybir.dt.float32,
                "bf16": mybir.dt.bfloat16}[self.mm_dtype]


def build_nc(cfg: Cfg):
    f32 = mybir.dt.float32
    mdt = cfg.mdt
    S, D, F, Dh = cfg.S, cfg.D, cfg.F, cfg.Dh
    QCH = cfg.qch
    Hc, nt_d, nt_f, qpc = cfg.Hc, cfg.nt_d, cfg.nt_f, cfg.qt_per_ch

    nc = bacc.Bacc("TRN2", target_bir_lowering=False, debug=False,
                   num_devices=cfg.n_cores)

    xT = nc.dram_tensor("xT", [D, S], mdt, kind="ExternalInput").ap()
    wqT = nc.dram_tensor("wqT", [D, F], mdt, kind="ExternalInput").ap()
    wkT = nc.dram_tensor("wkT", [D, F], mdt, kind="ExternalInput").ap()
    wvT = nc.dram_tensor("wvT", [D, F], mdt, kind="ExternalInput").ap()
    woT = nc.dram_tensor("woT", [F, D], mdt, kind="ExternalInput").ap()
    # padk[k, st] = 0.0 for padded keys else 1.0; padding is enforced by
    # zeroing padded keys' V rows and ones-column entries (numerator and
    # denominator both drop them), so exp needs no per-tile bias operand.
    padk = nc.dram_tensor("padk", [P, cfg.nt_s], f32, kind="ExternalInput").ap()
    out = nc.dram_tensor("out", [S, D], f32, kind="ExternalOutput").ap()

    Exp = mybir.ActivationFunctionType.Exp
    mult = mybir.AluOpType.mult

    with tile.TileContext(nc) as tc:
        with tc.tile_pool(name="sb_misc", bufs=1) as sb_misc:
            # --- constants ---
            pk = sb_misc.tile([P, cfg.nt_s], f32, tag="padk")
            nc.sync.dma_start(pk[:], padk)
            # triangular keep-mask in [k(part), q(free)] coords: 1 where q>=k
            tri_f = sb_misc.tile([P, P], f32, tag="tri_f")
            nc.gpsimd.memset(tri_f[:], 1.0)
            nc.gpsimd.affine_select(
                out=tri_f[:], in_=tri_f[:],
                compare_op=mybir.AluOpType.is_ge, fill=0.0,
                base=0, channel_multiplier=-1, pattern=[[1, P]],
            )
            tri = sb_misc.tile([P, P], mdt, tag="tri")
            nc.vector.tensor_copy(tri[:], tri_f[:])

            for _rep in range(getattr(cfg, "reps", 1)):
              with (
                # psA: double-wide score tiles (2 banks each); psB: pav;
                # psC: projection / output-projection accumulators.
                tc.tile_pool(name=f"psA{_rep}", bufs=2, space="PSUM") as psA,
                tc.tile_pool(name=f"psB{_rep}", bufs=2, space="PSUM") as psB,
                tc.tile_pool(name=f"psC{_rep}", bufs=2, space="PSUM") as psC,
                tc.tile_pool(name=f"sb_qT{_rep}", bufs=nt_f) as sb_qT,
                tc.tile_pool(name=f"sb_kT{_rep}", bufs=nt_f) as sb_kT,
                tc.tile_pool(name=f"sb_v{_rep}", bufs=cfg.nt_s) as sb_v,
                tc.tile_pool(name=f"sb_xt{_rep}", bufs=2 * nt_d) as sb_xt,
                tc.tile_pool(name=f"sb_w{_rep}", bufs=3 * nt_d) as sb_w,
                tc.tile_pool(name=f"sb_wo{_rep}", bufs=nt_f) as sb_wo,
                tc.tile_pool(name=f"sb_ctx{_rep}", bufs=nt_f) as sb_ctx,
                tc.tile_pool(name=f"sb_exp{_rep}", bufs=5) as sb_exp,
                tc.tile_pool(name=f"sb_out{_rep}", bufs=3) as sb_out,
                tc.tile_pool(name=f"sb_rc{_rep}", bufs=3) as sb_rc,
              ):
                qT_t = [sb_qT.tile([P, S], mdt, tag="qT", name="qT")
                        for _ in range(nt_f)]
                kT_t = [sb_kT.tile([P, S], mdt, tag="kT", name="kT")
                        for _ in range(nt_f)]
                v_t = [sb_v.tile([P, Hc * 2 * Dh], mdt, tag="v", name="v")
                       for _ in range(cfg.nt_s)]
                ctxT_t = [sb_ctx.tile([P, S], mdt, tag="ctxT", name="ctxT")
                          for _ in range(nt_f)]

                def _wload(wdram):
                    lst = []
                    for d in range(nt_d):
                        t = sb_w.tile([P, F], mdt, tag="w", name="w")
                        nc.sync.dma_start(t[:], wdram[d * P:(d + 1) * P, :])
                        lst.append(t)
                    return lst

                xts = {}

                def _xload(c):
                    lst = []
                    for d in range(nt_d):
                        t = sb_xt.tile([P, QCH], mdt, tag="xt", name="xt")
                        nc.sync.dma_start(
                            t[:], xT[d * P:(d + 1) * P, c * QCH:(c + 1) * QCH])
                        lst.append(t)
                    xts[c] = lst

                # first-needed first: wq + x0 interleaved in d order and
                # split in column halves, so the d=0.. tiles of BOTH land
                # progressively and the first psum group can start ~3us in
                wq_t, x0_t = [], []
                for d in range(nt_d):
                    tw = sb_w.tile([P, F], mdt, tag="w", name="w")
                    tx = sb_xt.tile([P, QCH], mdt, tag="xt", name="xt")
                    for s_ in (slice(0, F // 2), slice(F // 2, F)):
                        nc.sync.dma_start(tw[:, s_], wqT[d * P:(d + 1) * P, s_])
                    for s_ in (slice(0, QCH // 2), slice(QCH // 2, QCH)):
                        nc.sync.dma_start(tx[:, s_], xT[d * P:(d + 1) * P, s_])
                    wq_t.append(tw)
                    x0_t.append(tx)
                xts[0] = x0_t
                wk_t = _wload(wkT)
                wv_t = _wload(wvT)
                _xload(1)
                wo_t = []
                for f in range(nt_f):
                    t = sb_wo.tile([P, D], mdt, tag="wo")
                    nc.sync.dma_start(t[:], woT[f * P:(f + 1) * P, :])
                    wo_t.append(t)

                # ---- filler generators: one instruction per yield ----
                def gen_qk(wt, dstT, m, c):
                    ps = psC.tile([P, QCH], f32, tag="psC", name="ps")
                    for d in range(nt_d):
                        nc.tensor.matmul(
                            ps[:], wt[d][:, m * P:(m + 1) * P], xts[c][d][:],
                            start=(d == 0), stop=(d == nt_d - 1))
                        yield
                    nc.vector.tensor_copy(
                        dstT[m][:, c * QCH:(c + 1) * QCH], ps[:])
                    yield

                def gen_v(c, u):
                    st = c * qpc + u
                    ps = psC.tile([P, F], f32, tag="psC", name="ps")
                    for d in range(nt_d):
                        nc.tensor.matmul(
                            ps[:], xts[c][d][:, u * P:(u + 1) * P], wv_t[d][:],
                            start=(d == 0), stop=(d == nt_d - 1))
                        yield
                    # per head: [64 V features | 64 copies of the pad-keep
                    # flag]. The 64 ones-ish columns make the AV matmul
                    # produce the softmax denominator REPLICATED on psum
                    # partitions 64..127 (AV cost is per streamed column, so
                    # the wider stationary is free) -- the normalize multiply
                    # can then read a plain DVE reciprocal, no partition
                    # broadcast needed. Padded keys carry 0 here AND in their
                    # V rows, which drops them from numerator + denominator.
                    dst = v_t[st][:].rearrange("p (h e) -> p h e", e=2 * Dh)
                    nc.vector.tensor_scalar_mul(
                        dst[:, :, 0:Dh],
                        ps[:].rearrange("p (h e) -> p h e", e=Dh),
                        pk[:, st:st + 1])
                    yield
                    nc.vector.tensor_copy(
                        dst[:, :, Dh:2 * Dh],
                        pk[:, None, st:st + 1].to_broadcast([P, Hc, Dh]))
                    yield

                def gen_wo(c, u):
                    st = c * qpc + u
                    ot = sb_out.tile([P, D], f32, tag="ot", name="ot")
                    dw = 512
                    for dch in range(D // dw):
                        pwo = psC.tile([P, dw], f32, tag="psC", name="pwo")
                        for f2 in range(nt_f):
                            nc.tensor.matmul(
                                pwo[:],
                                ctxT_t[f2][:, st * P:(st + 1) * P],
                                wo_t[f2][:, dch * dw:(dch + 1) * dw],
                                start=(f2 == 0), stop=(f2 == nt_f - 1))
                            yield
                        nc.vector.tensor_copy(
                            ot[:, dch * dw:(dch + 1) * dw], pwo[:])
                        yield
                    nc.sync.dma_start(out[st * P:(st + 1) * P, :], ot[:])
                    yield

                def chain(gens):
                    for g in gens:
                        yield from g

                def proj_gens(c):
                    gens = []
                    for wt, dstT in ((wq_t, qT_t), (wk_t, kT_t)):
                        for m in range(nt_f):
                            gens.append(gen_qk(wt, dstT, m, c))
                    for u in range(qpc):
                        gens.append(gen_v(c, u))
                    return gens

                PROJ_STEPS = 2 * nt_f * (nt_d + 1) + qpc * (nt_d + 2)
                WO_STEPS = qpc * (2 * (nt_f + 1) + 1)

                _DONE = object()

                def attn_chunk(c, filler, nfill):
                    ktiles = qpc * (c + 1)
                    nsub = qpc * c              # full-width sub-diagonal tiles
                    points = Hc * ktiles
                    rate = nfill / points if points else 0.0
                    acc = 0.0

                    def inject():
                        nonlocal acc
                        acc += rate
                        while acc >= 1.0:
                            if next(filler, _DONE) is _DONE:
                                acc = 0.0
                                return
                            acc -= 1.0

                    deferred_norm = []

                    def flush_norm():
                        while deferred_norm:
                            deferred_norm.pop(0)()

                    # flat cross-head stream of score-pair units so head
                    # h+1's first exp overlaps head h's last AVs
                    units = []
                    for h in range(Hc):
                        t = 0
                        while t < ktiles:
                            n = 2 if t + 1 < nsub else 1
                            units.append((h, t, n))
                            t += n

                    pavs = {}
                    pend = []

                    def score_pair(h, t0, n):
                        f, r = divmod(h, cfg.heads_per_ft)
                        rows = slice(r * Dh, (r + 1) * Dh)
                        pss = psA.tile([P, 2 * QCH], f32, tag="psA",
                                       name="pss")
                        et = sb_exp.tile([P, 2 * QCH], mdt, tag="exp")
                        col0 = 0
                        for i in range(n):
                            t = t0 + i
                            col0 = max(0, (t - nsub) * P)  # last may be diag
                            nc.tensor.matmul(
                                pss[:, i * QCH + col0:(i + 1) * QCH],
                                kT_t[f][rows, t * P:(t + 1) * P],
                                qT_t[f][rows, c * QCH + col0:(c + 1) * QCH],
                                start=True, stop=True,
                                tile_position=(r * Dh, 0))
                        lo = col0 if n == 1 else 0
                        nc.scalar.activation(
                            et[:, lo:(n - 1) * QCH + QCH],
                            pss[:, lo:(n - 1) * QCH + QCH], Exp,
                            scale=float(Dh) ** -0.5)
                        for i in range(n):
                            t = t0 + i
                            col0 = max(0, (t - nsub) * P)
                            if t >= nsub:
                                nc.vector.tensor_tensor(
                                    et[:, i * QCH + col0:i * QCH + col0 + P],
                                    et[:, i * QCH + col0:i * QCH + col0 + P],
                                    tri[:], mult)
                            pend.append((h, t, col0, et, i))

                    def av():
                        h, t, col0, et, i = pend.pop(0)
                        if t == 0:
                            pavs[h] = psB.tile([P, QCH], f32, tag="pav",
                                               name="pav")
                        pav = pavs[h]
                        nc.tensor.matmul(
                            pav[:, col0:],
                            v_t[t][:, h * 2 * Dh:(h + 1) * 2 * Dh],
                            et[:, i * QCH + col0:(i + 1) * QCH],
                            start=(t == 0), stop=(t == ktiles - 1))
                        inject()
                        if t == ktiles - 1:
                            f, r = divmod(h, cfg.heads_per_ft)
                            rows = slice(r * Dh, (r + 1) * Dh)

                            def norm(f=f, rows=rows, pav=pav, h=h):
                                rcb = sb_rc.tile([Dh, QCH], f32, tag="rcb")
                                nc.vector.reciprocal(
                                    rcb[:], pav[Dh:2 * Dh, :])
                                nc.vector.tensor_tensor(
                                    ctxT_t[f][rows, c * QCH:(c + 1) * QCH],
                                    pav[0:Dh, :], rcb[:], mult)
                                del pavs[h]
                            deferred_norm.append(norm)

                    for (h, t0, n) in units:
                        score_pair(h, t0, n)
                        flush_norm()
                        while len(pend) > LOOKAHEAD:
                            av()
                    while pend:
                        av()
                    flush_norm()
                    # drain leftover filler
                    while next(filler, _DONE) is not _DONE:
                        pass

                # ---- pipelined schedule ----
                # proj(0); attn(c)+proj(c+1) for c<3; attn(3) gets ALL the
                # deferred Wo chunks (its exp load has no proj filler left)
                for _ in chain(proj_gens(0)):
                    pass
                for c in range(cfg.nqc):
                    if c + 2 < cfg.nqc:
                        _xload(c + 2)
                    gens, nfill = [], 0
                    if c + 1 < cfg.nqc:
                        gens += proj_gens(c + 1)
                        nfill += PROJ_STEPS
                    else:
                        for cc in range(cfg.nqc - 1):
                            gens += [gen_wo(cc, u) for u in range(qpc)]
                            nfill += WO_STEPS
                    attn_chunk(c, chain(gens), nfill)
                for g in [gen_wo(cfg.nqc - 1, u) for u in range(qpc)]:
                    for _ in g:
                        pass

    nc.compile()
    return nc


_NC_CACHE = {}


def _get_nc(cfg: Cfg):
    key = (cfg.B, cfg.S, cfg.D, cfg.H, cfg.n_cores, cfg.qch, cfg.mm_dtype,
           cfg.reps)
    if key not in _NC_CACHE:
        _NC_CACHE[key] = build_nc(cfg)
    return _NC_CACHE[key]


def make_in_maps(cfg: Cfg, x_self, padding_mask, Wq, Wk, Wv, Wo):
    """Host-side sharding: slice + transpose per core."""
    np_mdt = mybir.dt.np(cfg.mdt)
    if cfg.mm_dtype == "fp32r":
        rnd = _round_f32r
    else:
        def rnd(a):
            return np.ascontiguousarray(a, dtype=np.float32).astype(np_mdt)
    in_maps = []
    for core in range(cfg.n_cores):
        b, g = divmod(core, cfg.groups)
        fsl = slice(g * cfg.F, (g + 1) * cfg.F)
        keep = np.where(padding_mask[b], np.float32(0.0), np.float32(1.0))
        in_maps.append({
            "xT": rnd(x_self[b].T),
            "wqT": rnd(Wq[fsl, :].T),
            "wkT": rnd(Wk[fsl, :].T),
            "wvT": rnd(Wv[fsl, :].T),
            "woT": rnd(Wo[:, fsl].T),
            "padk": np.ascontiguousarray(
                keep.reshape(cfg.nt_s, P).T).astype(np.float32),
        })
    return in_maps


def kernel(x_self, x_other, padding_mask, Wq, Wk, Wv, Wo, _trace=False):
    x_self = np.asarray(x_self, dtype=np.float32)
    padding_mask = np.asarray(padding_mask)
    Wq = np.asarray(Wq, dtype=np.float32)
    Wk = np.asarray(Wk, dtype=np.float32)
    Wv = np.asarray(Wv, dtype=np.float32)
    Wo = np.asarray(Wo, dtype=np.float32)

    B, S, D = x_self.shape
    cfg = Cfg(B=B, S=S, D=D)
    nc = _get_nc(cfg)
    in_maps = make_in_maps(cfg, x_self, padding_mask, Wq, Wk, Wv, Wo)
    res = run_bass_kernel_spmd(
        nc, in_maps, core_ids=list(range(cfg.n_cores)), trace=_trace)

    out = np.zeros((B, S, D), dtype=np.float32)
    for core in range(cfg.n_cores):
        b = core // cfg.groups
        out[b] += res.results[core]["out"]
    if _trace:
        kernel.last_exec_time_ns = res.exec_time_ns
        kernel.last_results = res
    return out


# revision 35
# speedup vs baseline: 1.0850x; 1.0850x over previous
"""Multi-head causal self-attention on 8 Trainium2 NeuronCores.

Problem: B=4, S=2048, D=1024, H=16 heads (Dh=64), fp32, causal + key-padding
mask, out = softmax(mask(QK^T/sqrt(Dh))) V Wo^T with Q/K/V = x @ W*^T.

Sharding (data-parallel over batch x tensor-parallel over heads):
  core = 2*b + g  (b in 0..3, g in 0..1): batch b, head group g (8 heads).
  Each core computes its 8 heads' attention and a partial output projection
  through its row-slice of Wo; the host sums the two partials per batch
  (the "all-reduce" of the hint, done on host since outputs are gathered
  anyway).

Per-core kernel, one software-pipelined loop over 512-column s-chunks:
  - All matmul operands are bf16 (psum accumulation f32): same PE rate as
    fp32r at >=256-wide output, but no 4x penalty on the narrow diagonal
    matmuls, half the input DMA, and the 16-bit DVE fast path for the
    triangular mask multiply.
  - Causality makes attention q-chunk c depend only on projection chunks
    0..c, so the schedule is: proj(0); attn(0)+proj(1); attn(1)+proj(2)+
    Wo(0); attn(2)+proj(3)+Wo(1); attn(3)+Wo(2); Wo(3).
  - The attention inner loop is Activation-(exp-)bound: per 128x512 score
    tile the PE owes 426ns (QK^T + AV) while exp costs ~550ns. Projection /
    output-projection matmuls are therefore injected one instruction at a
    time (generator fillers) after each AV, and scores are emitted
    LOOKAHEAD tiles ahead of their AVs so the in-order PE queue never
    blocks on exp latency.
  - scores computed TRANSPOSED per head: s^T[k, q] = k^T_tile.T @ q^T so the
    softmaxed tile feeds the AV matmul directly as the moving operand.
  - sub-diagonal k-tile PAIRS share one [128, 1024] psum tile and ONE exp
    instruction (fewer Activation fixed overheads); the padding mask lives
    in V (zeroed rows + pad-keep columns), not in an exp bias operand.
  - V carries 64 pad-keep (ones) columns per head, so the AV matmul yields
    the softmax denominator REPLICATED on psum partitions 64..127 (AV cost
    is per streamed column — the wide stationary is free): normalize is a
    plain DVE reciprocal + multiply into ctx^T, no partition broadcast.
  - ctx^T tiles are the stationary operand of the output projection
    out[s, d] = ctx^T.T @ Wo_slice^T; outputs are stored bf16 (the host
    sums the two partial outputs per batch in f32), halving output DMA --
    aggregate HBM bandwidth across the 8 cores is the one resource the
    cost model underestimates, so DMA bytes matter on real hardware.
"""

import numpy as np

import concourse.bass as bass
import concourse.mybir as mybir
import concourse.tile as tile
from concourse import bacc
from concourse.bass_utils import run_bass_kernel_spmd

P = 128
NEG = -1.0e30
LOOKAHEAD = 3


def _round_f32r(a: np.ndarray) -> np.ndarray:
    """Round fp32 values to the PE's fp32r grid (11-bit mantissa,
    round-half-to-even at bit 12)."""
    bits = np.ascontiguousarray(a, dtype=np.float32).view(np.uint32)
    low = bits & np.uint32(0xFFF)
    hi = bits & np.uint32(0xFFFFF000)
    add = (low > 0x800) | ((low == 0x800) & (((bits >> 12) & 1) == 1))
    return (hi + (add.astype(np.uint32) << 12)).view(np.float32)


class Cfg:
    def __init__(self, B=4, S=2048, D=1024, H=16, Dh=64, n_cores=8, qch=512,
                 mm_dtype="bf16", out_dtype="bf16", reps=1):
        self.out_dtype = out_dtype
        self.reps = reps
        self.B, self.S, self.D, self.H, self.Dh = B, S, D, H, Dh
        self.n_cores = n_cores
        self.groups = n_cores // B              # head groups (tensor-parallel)
        self.Hc = H // self.groups              # heads per core
        self.F = self.Hc * Dh                   # per-core q/k/v feature width
        self.qch = qch                          # q columns per score matmul
        self.nqc = S // qch                     # q chunks
        self.qt_per_ch = qch // P               # 128-row q tiles per chunk
        self.nt_s = S // P                      # key/seq tiles
        self.nt_d = D // P                      # contraction tiles (D)
        self.nt_f = self.F // P                 # feature tiles
        self.heads_per_ft = P // Dh             # heads packed per feature tile
        self.mm_dtype = mm_dtype

    @property
    def mdt(self):
        return {"fp32r": mybir.dt.float32r,
                "fp32": mybir.dt.float32,
                "bf16": mybir.dt.bfloat16}[self.mm_dtype]


def build_nc(cfg: Cfg):
    f32 = mybir.dt.float32
    mdt = cfg.mdt
    S, D, F, Dh = cfg.S, cfg.D, cfg.F, cfg.Dh
    QCH = cfg.qch
    Hc, nt_d, nt_f, qpc = cfg.Hc, cfg.nt_d, cfg.nt_f, cfg.qt_per_ch

    nc = bacc.Bacc("TRN2", target_bir_lowering=False, debug=False,
                   num_devices=cfg.n_cores)

    xT = nc.dram_tensor("xT", [D, S], mdt, kind="ExternalInput").ap()
    wqT = nc.dram_tensor("wqT", [D, F], mdt, kind="ExternalInput").ap()
    wkT = nc.dram_tensor("wkT", [D, F], mdt, kind="ExternalInput").ap()
    wvT = nc.dram_tensor("wvT", [D, F], mdt, kind="ExternalInput").ap()
    woT = nc.dram_tensor("woT", [F, D], mdt, kind="ExternalInput").ap()
    # padk[k, st] = 0.0 for padded keys else 1.0; padding is enforced by
    # zeroing padded keys' V rows and ones-column entries (numerator and
    # denominator both drop them), so exp needs no per-tile bias operand.
    padk = nc.dram_tensor("padk", [P, cfg.nt_s], f32, kind="ExternalInput").ap()
    odt = f32 if cfg.out_dtype == "f32" else mybir.dt.bfloat16
    out = nc.dram_tensor("out", [S, D], odt, kind="ExternalOutput").ap()

    Exp = mybir.ActivationFunctionType.Exp
    mult = mybir.AluOpType.mult

    with tile.TileContext(nc) as tc:
        with tc.tile_pool(name="sb_misc", bufs=1) as sb_misc:
            # --- constants ---
            pk = sb_misc.tile([P, cfg.nt_s], f32, tag="padk")
            nc.sync.dma_start(pk[:], padk)
            # triangular keep-mask in [k(part), q(free)] coords: 1 where q>=k
            tri_f = sb_misc.tile([P, P], f32, tag="tri_f")
            nc.gpsimd.memset(tri_f[:], 1.0)
            nc.gpsimd.affine_select(
                out=tri_f[:], in_=tri_f[:],
                compare_op=mybir.AluOpType.is_ge, fill=0.0,
                base=0, channel_multiplier=-1, pattern=[[1, P]],
            )
            tri = sb_misc.tile([P, P], mdt, tag="tri")
            nc.vector.tensor_copy(tri[:], tri_f[:])

            for _rep in range(getattr(cfg, "reps", 1)):
              with (
                # psA: double-wide score tiles (2 banks each); psB: pav;
                # psC: projection / output-projection accumulators.
                tc.tile_pool(name=f"psA{_rep}", bufs=2, space="PSUM") as psA,
                tc.tile_pool(name=f"psB{_rep}", bufs=2, space="PSUM") as psB,
                tc.tile_pool(name=f"psC{_rep}", bufs=2, space="PSUM") as psC,
                tc.tile_pool(name=f"sb_qT{_rep}", bufs=nt_f) as sb_qT,
                tc.tile_pool(name=f"sb_kT{_rep}", bufs=nt_f) as sb_kT,
                tc.tile_pool(name=f"sb_v{_rep}", bufs=cfg.nt_s) as sb_v,
                tc.tile_pool(name=f"sb_xt{_rep}", bufs=2 * nt_d) as sb_xt,
                tc.tile_pool(name=f"sb_w{_rep}", bufs=3 * nt_d) as sb_w,
                tc.tile_pool(name=f"sb_wo{_rep}", bufs=nt_f) as sb_wo,
                tc.tile_pool(name=f"sb_ctx{_rep}", bufs=nt_f) as sb_ctx,
                tc.tile_pool(name=f"sb_exp{_rep}", bufs=6) as sb_exp,
                tc.tile_pool(name=f"sb_out{_rep}", bufs=3) as sb_out,
                tc.tile_pool(name=f"sb_rc{_rep}", bufs=3) as sb_rc,
              ):
                qT_t = [sb_qT.tile([P, S], mdt, tag="qT", name="qT")
                        for _ in range(nt_f)]
                kT_t = [sb_kT.tile([P, S], mdt, tag="kT", name="kT")
                        for _ in range(nt_f)]
                v_t = [sb_v.tile([P, Hc * 2 * Dh], mdt, tag="v", name="v")
                       for _ in range(cfg.nt_s)]
                ctxT_t = [sb_ctx.tile([P, S], mdt, tag="ctxT", name="ctxT")
                          for _ in range(nt_f)]

                def _wload(wdram):
                    lst = []
                    for d in range(nt_d):
                        t = sb_w.tile([P, F], mdt, tag="w", name="w")
                        nc.sync.dma_start(t[:], wdram[d * P:(d + 1) * P, :])
                        lst.append(t)
                    return lst

                xts = {}

                def _xload(c):
                    lst = []
                    for d in range(nt_d):
                        t = sb_xt.tile([P, QCH], mdt, tag="xt", name="xt")
                        nc.sync.dma_start(
                            t[:], xT[d * P:(d + 1) * P, c * QCH:(c + 1) * QCH])
                        lst.append(t)
                    xts[c] = lst

                # first-needed first. The m=0/d=0 matmul of proj(0) needs
                # only wq[0] cols 0:128 and x0[0]; split those two DMAs so
                # the first psum group starts ~4us earlier, and interleave
                # wq/x0 issue order so the d-tiles land progressively.
                wq_t, x0_t = [], []
                for d in range(nt_d):
                    tw = sb_w.tile([P, F], mdt, tag="w", name="w")
                    tx = sb_xt.tile([P, QCH], mdt, tag="xt", name="xt")
                    if d == 0:
                        for s_ in (slice(0, P), slice(P, F)):
                            nc.sync.dma_start(tw[:, s_], wqT[0:P, s_])
                        for s_ in (slice(0, QCH // 2), slice(QCH // 2, QCH)):
                            nc.sync.dma_start(tx[:, s_], xT[0:P, s_])
                    else:
                        nc.sync.dma_start(tw[:], wqT[d * P:(d + 1) * P, :])
                        nc.sync.dma_start(
                            tx[:], xT[d * P:(d + 1) * P, 0:QCH])
                    wq_t.append(tw)
                    x0_t.append(tx)
                xts[0] = x0_t
                wk_t = _wload(wkT)
                wv_t = _wload(wvT)
                _xload(1)
                wo_t = []
                for f in range(nt_f):
                    t = sb_wo.tile([P, D], mdt, tag="wo")
                    nc.sync.dma_start(t[:], woT[f * P:(f + 1) * P, :])
                    wo_t.append(t)

                # ---- filler generators: one instruction per yield ----
                def gen_qk(wt, dstT, m, c):
                    ps = psC.tile([P, QCH], f32, tag="psC", name="ps")
                    for d in range(nt_d):
                        nc.tensor.matmul(
                            ps[:], wt[d][:, m * P:(m + 1) * P], xts[c][d][:],
                            start=(d == 0), stop=(d == nt_d - 1))
                        yield
                    nc.vector.tensor_copy(
                        dstT[m][:, c * QCH:(c + 1) * QCH], ps[:])
                    yield

                def gen_v(c, u):
                    st = c * qpc + u
                    ps = psC.tile([P, F], f32, tag="psC", name="ps")
                    for d in range(nt_d):
                        nc.tensor.matmul(
                            ps[:], xts[c][d][:, u * P:(u + 1) * P], wv_t[d][:],
                            start=(d == 0), stop=(d == nt_d - 1))
                        yield
                    # per head: [64 V features | 64 copies of the pad-keep
                    # flag]. The 64 ones-ish columns make the AV matmul
                    # produce the softmax denominator REPLICATED on psum
                    # partitions 64..127 (AV cost is per streamed column, so
                    # the wider stationary is free) -- the normalize multiply
                    # can then read a plain DVE reciprocal, no partition
                    # broadcast needed. Padded keys carry 0 here AND in their
                    # V rows, which drops them from numerator + denominator.
                    dst = v_t[st][:].rearrange("p (h e) -> p h e", e=2 * Dh)
                    nc.vector.tensor_scalar_mul(
                        dst[:, :, 0:Dh],
                        ps[:].rearrange("p (h e) -> p h e", e=Dh),
                        pk[:, st:st + 1])
                    yield
                    nc.vector.tensor_copy(
                        dst[:, :, Dh:2 * Dh],
                        pk[:, None, st:st + 1].to_broadcast([P, Hc, Dh]))
                    yield

                def gen_wo(c, u):
                    st = c * qpc + u
                    ot = sb_out.tile([P, D], odt, tag="ot", name="ot")
                    dw = 512
                    for dch in range(D // dw):
                        pwo = psC.tile([P, dw], f32, tag="psC", name="pwo")
                        for f2 in range(nt_f):
                            nc.tensor.matmul(
                                pwo[:],
                                ctxT_t[f2][:, st * P:(st + 1) * P],
                                wo_t[f2][:, dch * dw:(dch + 1) * dw],
                                start=(f2 == 0), stop=(f2 == nt_f - 1))
                            yield
                        nc.vector.tensor_copy(
                            ot[:, dch * dw:(dch + 1) * dw], pwo[:])
                        yield
                        # per-half DMA so the dch=0 store overlaps the
                        # dch=1 matmuls/copy (shrinks the final-tile drain)
                        nc.sync.dma_start(
                            out[st * P:(st + 1) * P, dch * dw:(dch + 1) * dw],
                            ot[:, dch * dw:(dch + 1) * dw])
                        yield

                def chain(gens):
                    for g in gens:
                        yield from g

                def proj_gens(c):
                    gens = []
                    for wt, dstT in ((wq_t, qT_t), (wk_t, kT_t)):
                        for m in range(nt_f):
                            gens.append(gen_qk(wt, dstT, m, c))
                    for u in range(qpc):
                        gens.append(gen_v(c, u))
                    return gens

                PROJ_STEPS = 2 * nt_f * (nt_d + 1) + qpc * (nt_d + 2)
                WO_STEPS = qpc * 2 * (nt_f + 2)

                _DONE = object()

                def attn_chunk(c, filler, nfill):
                    ktiles = qpc * (c + 1)
                    nsub = qpc * c              # full-width sub-diagonal tiles
                    points = Hc * ktiles
                    rate = nfill / points if points else 0.0
                    acc = 0.0

                    def inject():
                        nonlocal acc
                        acc += rate
                        while acc >= 1.0:
                            if next(filler, _DONE) is _DONE:
                                acc = 0.0
                                return
                            acc -= 1.0

                    deferred_norm = []

                    def flush_norm():
                        while deferred_norm:
                            deferred_norm.pop(0)()

                    # flat cross-head stream of score-pair units so head
                    # h+1's first exp overlaps head h's last AVs
                    units = []
                    for h in range(Hc):
                        t = 0
                        while t < ktiles:
                            n = 2 if t + 1 < nsub else 1
                            units.append((h, t, n))
                            t += n

                    pavs = {}
                    pend = []

                    def score_pair(h, t0, n):
                        f, r = divmod(h, cfg.heads_per_ft)
                        rows = slice(r * Dh, (r + 1) * Dh)
                        pss = psA.tile([P, 2 * QCH], f32, tag="psA",
                                       name="pss")
                        et = sb_exp.tile([P, 2 * QCH], mdt, tag="exp")
                        col0 = 0
                        for i in range(n):
                            t = t0 + i
                            col0 = max(0, (t - nsub) * P)  # last may be diag
                            nc.tensor.matmul(
                                pss[:, i * QCH + col0:(i + 1) * QCH],
                                kT_t[f][rows, t * P:(t + 1) * P],
                                qT_t[f][rows, c * QCH + col0:(c + 1) * QCH],
                                start=True, stop=True,
                                tile_position=(r * Dh, 0))
                        lo = col0 if n == 1 else 0
                        nc.scalar.activation(
                            et[:, lo:(n - 1) * QCH + QCH],
                            pss[:, lo:(n - 1) * QCH + QCH], Exp,
                            scale=float(Dh) ** -0.5)
                        for i in range(n):
                            t = t0 + i
                            col0 = max(0, (t - nsub) * P)
                            if t >= nsub:
                                nc.vector.tensor_tensor(
                                    et[:, i * QCH + col0:i * QCH + col0 + P],
                                    et[:, i * QCH + col0:i * QCH + col0 + P],
                                    tri[:], mult)
                            pend.append((h, t, col0, et, i))

                    def av():
                        h, t, col0, et, i = pend.pop(0)
                        if t == 0:
                            pavs[h] = psB.tile([P, QCH], f32, tag="pav",
                                               name="pav")
                        pav = pavs[h]
                        nc.tensor.matmul(
                            pav[:, col0:],
                            v_t[t][:, h * 2 * Dh:(h + 1) * 2 * Dh],
                            et[:, i * QCH + col0:(i + 1) * QCH],
                            start=(t == 0), stop=(t == ktiles - 1))
                        inject()
                        if t == ktiles - 1:
                            f, r = divmod(h, cfg.heads_per_ft)
                            rows = slice(r * Dh, (r + 1) * Dh)

                            def norm(f=f, rows=rows, pav=pav, h=h):
                                rcb = sb_rc.tile([Dh, QCH], f32, tag="rcb")
                                nc.vector.reciprocal(
                                    rcb[:], pav[Dh:2 * Dh, :])
                                nc.vector.tensor_tensor(
                                    ctxT_t[f][rows, c * QCH:(c + 1) * QCH],
                                    pav[0:Dh, :], rcb[:], mult)
                                del pavs[h]
                            deferred_norm.append(norm)

                    for (h, t0, n) in units:
                        score_pair(h, t0, n)
                        flush_norm()
                        while len(pend) > LOOKAHEAD:
                            av()
                    while pend:
                        av()
                    flush_norm()
                    # drain leftover filler
                    while next(filler, _DONE) is not _DONE:
                        pass

                # ---- pipelined schedule ----
                # proj(0); attn(c)+proj(c+1) for c<3; attn(3) gets ALL the
                # deferred Wo chunks (its exp load has no proj filler left)
                for _ in chain(proj_gens(0)):
                    pass
                for c in range(cfg.nqc):
                    if c + 2 < cfg.nqc:
                        _xload(c + 2)
                    gens, nfill = [], 0
                    if c + 1 < cfg.nqc:
                        gens += proj_gens(c + 1)
                        nfill += PROJ_STEPS
                    else:
                        for cc in range(cfg.nqc - 1):
                            gens += [gen_wo(cc, u) for u in range(qpc)]
                            nfill += WO_STEPS
                    attn_chunk(c, chain(gens), nfill)
                for g in [gen_wo(cfg.nqc - 1, u) for u in range(qpc)]:
                    for _ in g:
                        pass

    nc.compile()
    return nc


_NC_CACHE = {}


def _get_nc(cfg: Cfg):
    key = (cfg.B, cfg.S, cfg.D, cfg.H, cfg.n_cores, cfg.qch, cfg.mm_dtype,
           cfg.out_dtype, cfg.reps)
    if key not in _NC_CACHE:
        _NC_CACHE[key] = build_nc(cfg)
    return _NC_CACHE[key]


def make_in_maps(cfg: Cfg, x_self, padding_mask, Wq, Wk, Wv, Wo):
    """Host-side sharding: slice + transpose per core."""
    np_mdt = mybir.dt.np(cfg.mdt)
    if cfg.mm_dtype == "fp32r":
        rnd = _round_f32r
    else:
        def rnd(a):
            return np.ascontiguousarray(a, dtype=np.float32).astype(np_mdt)
    in_maps = []
    for core in range(cfg.n_cores):
        b, g = divmod(core, cfg.groups)
        fsl = slice(g * cfg.F, (g + 1) * cfg.F)
        keep = np.where(padding_mask[b], np.float32(0.0), np.float32(1.0))
        in_maps.append({
            "xT": rnd(x_self[b].T),
            "wqT": rnd(Wq[fsl, :].T),
            "wkT": rnd(Wk[fsl, :].T),
            "wvT": rnd(Wv[fsl, :].T),
            "woT": rnd(Wo[:, fsl].T),
            "padk": np.ascontiguousarray(
                keep.reshape(cfg.nt_s, P).T).astype(np.float32),
        })
    return in_maps


def kernel(x_self, x_other, padding_mask, Wq, Wk, Wv, Wo, _trace=False):
    x_self = np.asarray(x_self, dtype=np.float32)
    padding_mask = np.asarray(padding_mask)
    Wq = np.asarray(Wq, dtype=np.float32)
    Wk = np.asarray(Wk, dtype=np.float32)
    Wv = np.asarray(Wv, dtype=np.float32)
    Wo = np.asarray(Wo, dtype=np.float32)

    B, S, D = x_self.shape
    cfg = Cfg(B=B, S=S, D=D)
    nc = _get_nc(cfg)
    in_maps = make_in_maps(cfg, x_self, padding_mask, Wq, Wk, Wv, Wo)
    res = run_bass_kernel_spmd(
        nc, in_maps, core_ids=list(range(cfg.n_cores)), trace=_trace)

    out = np.zeros((B, S, D), dtype=np.float32)
    for core in range(cfg.n_cores):
        b = core // cfg.groups
        out[b] += np.asarray(res.results[core]["out"], dtype=np.float32)
    if _trace:
        kernel.last_exec_time_ns = res.exec_time_ns
        kernel.last_results = res
    return out
